# revision 24
# baseline (speedup 1.0000x reference)
"""LSA attention (full S x S attention with diagonal self-exclusion) on 8 TRN2 cores.

Full inputs Q,K,V [4,12,2048,64] f32; heads flattened to 48 and split 6 per core
(no cross-core communication). Host-side prep: K,Q transposed to [h, 64, S] bf16
(KT/QT), V bf16; KT is pre-scaled by c = 2^23*log2(e)/(T*65536) so the fp32
scores in PSUM are directly in the Schraudolph-exp integer domain.

The softmax exp is the bottleneck (ACT engine: 1 elem/lane/cycle @1.2GHz), so it
is split across two engines per 512-wide q strip (16 k-blocks of 128):
  - k-blocks 0-11: ACT exps a whole [128, 1536] fp32 PSUM group tile in ONE
    activation (exp(sc * ln2/128) == exp(s/T)); 4 calls per strip amortize the
    ~350-cycle ACT instruction overhead.
  - k-blocks 12-14: DVE computes a Schraudolph exp: round(sc + B2) -> int16,
    bit-viewed as bf16 (~3.3% max rel err on 3/16 of the weights), one
    tensor_scalar instruction per group. k-block 15 stays on ACT (N=512 call).
Score matmuls are row-packed: KT/QT duplicated to partitions 64-127, adjacent
k-blocks run on PE array row groups 0/64 concurrently (contract dim is 64).
AV accumulates out^T[65,q] in PSUM via V' tiles carrying a ones column (row 64 =
softmax denominators). The diagonal is zeroed by a (1-I) mask multiply on DVE.
Strip tail: 4 PE transposes into one PSUM bank, reciprocal + scale on DVE, DMA
out. Software pipelining: scores(step k+1) issue before AV(step k-1) (AV lags
two steps) so neither ACT nor the in-order PE ever stalls at strip boundaries.
"""

import sys

for _p in ("/opt/trn_rl_repo",):
    if _p not in sys.path:
        sys.path.insert(0, _p)

import math

import ml_dtypes
import numpy as np

import concourse.bass as bass  # noqa: F401  (registers trn types)
import concourse.bacc as bacc
import concourse.mybir as mybir
import concourse.tile as tile
from concourse.bass_utils import run_bass_kernel_spmd
from concourse.masks import make_identity

N_CORES = 8
B, H, S, D = 4, 12, 2048, 64
HPC = (B * H) // N_CORES  # heads per core = 6
NKB = S // 128  # 16 k-blocks of 128
STRIP = 512
NSTRIP = S // STRIP  # 4 q strips per head
NQT = STRIP // 128  # 4 q-tiles per strip
# (group, engine): 'a' = ACT exp, 'v' = DVE two-phase Schraudolph exp.
# The DVE group leads the strip: its ~1.6us of ACT silence absorbs the PE's
# strip-boundary bunch (AV flush + transposes + next scores) without ACT gaps.
GRPS = [
    ((15,), "a"),
    ((12,), "v"),
    ((0, 1, 2), "a"),
    ((13,), "v"),
    ((3, 4, 5), "a"),
    ((14,), "v"),
    ((6, 7, 8), "a"),
    ((9, 10, 11), "a"),
]
AV_FIRST = 15  # first k-block accumulated into ot (start=True)
AV_LAST = 11  # last k-block accumulated (stop=True)
FP32 = mybir.dt.float32
BF16 = mybir.dt.bfloat16
I16 = mybir.dt.int16
EXP = mybir.ActivationFunctionType.Exp
# two-phase Schraudolph: round(u+B-128) and round(u+B-192) as int16, bit-viewed
# as bf16, estimate exp/2 and exp*2^-1.5 with error sawtooths a half-period
# apart; eta1 + sqrt(2)*eta2 averages them (max rel err ~1.0%)
SCHRAUD_C = 453000
B2 = (127 * (1 << 23) - SCHRAUD_C) / 65536.0
SQRT2 = math.sqrt(2.0)
ACT_SCALE = math.log(2.0) / 128.0  # undoes the host-side Schraudolph pre-scale


def build_nc():
    nc = bacc.Bacc(None, target_bir_lowering=False)
    qt_d = nc.dram_tensor("QT", [HPC, D, S], BF16, kind="ExternalInput")
    kt_d = nc.dram_tensor("KT", [HPC, D, S], BF16, kind="ExternalInput")
    v_d = nc.dram_tensor("V", [HPC, S, D], BF16, kind="ExternalInput")
    out_d = nc.dram_tensor("out", [HPC, S, D], FP32, kind="ExternalOutput")

    with tile.TileContext(nc) as tc:
        with (
            tc.tile_pool(name="consts", bufs=1) as constp,
            tc.tile_pool(name="tr", bufs=2) as trp,
            tc.tile_pool(name="vpool", bufs=2) as vpool,
            tc.tile_pool(name="expa", bufs=6) as expa,
            tc.tile_pool(name="expv", bufs=6) as expv,
            tc.tile_pool(name="expi", bufs=2) as expi,
            tc.tile_pool(name="otsb", bufs=2) as otp,
            tc.tile_pool(name="stage", bufs=2) as stgp,
            tc.tile_pool(name="small", bufs=2) as smallp,
            tc.tile_pool(name="ps_s", bufs=2, space="PSUM") as ps_s,
            tc.tile_pool(name="ps_o", bufs=1, space="PSUM") as ps_o,
            tc.tile_pool(name="ps_t", bufs=1, space="PSUM") as ps_t,
        ):
            ident = constp.tile([128, 128], FP32)
            make_identity(nc, ident[:])
            ome = constp.tile([128, 128], BF16)  # 1 - I, zeroes the diagonal
            nc.vector.memset(ome[:], 1.0)
            idb = constp.tile([128, 128], BF16)
            nc.vector.tensor_copy(idb[:], ident[:])
            nc.vector.tensor_sub(ome[:], ome[:], idb[:])
            # -6000*I: accumulated onto diagonal score blocks by the PE itself
            # (exp(sc-6000) underflows to ~0 in both the ACT and the
            # int16-Schraudolph path; no cross-engine mask edge remains)
            negbig = constp.tile([128, 128], BF16)
            nc.vector.tensor_scalar_mul(negbig[:], idb[:], -6000.0)

            # preload the exp table set (one-time ~2.7us, overlaps warmup)
            tldin = constp.tile([128, 1], FP32)
            tldout = constp.tile([128, 1], FP32)
            nc.vector.memset(tldin[:], 0.0)
            nc.scalar.activation(tldout[:], tldin[:], EXP)

            # PE warmup: ~5us of dummy matmuls (>= one 3.4us HAM window) so the
            # clock gate opens to K=8/8 during the head-0 DMAs
            wsrc = constp.tile([128, 256], BF16, tag="wsrc")
            nc.vector.memset(wsrc[:], 0.5)
            for _w in range(24):
                wt = ps_t.tile([128, 256], FP32, tag="tr")
                nc.tensor.matmul(wt[:], idb[:], wsrc[:], start=True, stop=True)

            head_tiles = {}

            def load_head(h):
                # KT/QT [64, S] bf16, duplicated to partitions 64-127 so the
                # odd k-blocks' score matmuls run on PE array row group 64
                kt2 = trp.tile([128, S], BF16, tag="kt")
                nc.sync.dma_start(kt2[0:64, :], kt_d[h])
                nc.sync.dma_start(kt2[64:128, :], kt_d[h])
                qt2 = trp.tile([128, S], BF16, tag="qt")
                nc.sync.dma_start(qt2[0:64, :], qt_d[h])
                nc.sync.dma_start(qt2[64:128, :], qt_d[h])
                # V' tiles [128, 65] per k-block: V rows + ones column
                vt = vpool.tile([128, NKB * (D + 1)], BF16, tag="vt")
                vt3 = vt.rearrange("p (n c) -> p n c", c=D + 1)
                nc.sync.dma_start(
                    vt3[:, :, 0:D], v_d[h].rearrange("(n p) d -> p n d", p=128)
                )
                nc.vector.memset(vt3[:, :, D : D + 1], 1.0)
                head_tiles[h] = (kt2, qt2, vt)

            load_head(0)

            steps = []
            for h in range(HPC):
                for st in range(NSTRIP):
                    for gi, (grp, eng) in enumerate(GRPS):
                        steps.append((h, st, gi, grp, eng))

            def issue_scores(h, st, grp):
                kt2, qt2, _ = head_tiles[h]
                q0 = st * STRIP
                sc = ps_s.tile([128, 3 * STRIP], FP32, tag="sc")
                for i, kb in enumerate(grp):
                    rg = 64 * (i % 2)  # alternate row groups -> concurrent MMs
                    nc.tensor.matmul(
                        sc[:, i * STRIP : (i + 1) * STRIP],
                        kt2[rg : rg + 64, kb * 128 : (kb + 1) * 128],
                        qt2[rg : rg + 64, q0 : q0 + STRIP],
                        start=True,
                        stop=True,
                        skip_group_check=True,
                    )
                    if q0 <= kb * 128 < q0 + STRIP:
                        off = i * STRIP + kb * 128 - q0
                        nc.tensor.matmul(
                            sc[:, off : off + 128],
                            idb[:],
                            negbig[:],
                            start=False,
                            stop=True,
                            skip_group_check=True,
                        )
                return sc

            def issue_exp(h, st, grp, eng, sc):
                q0 = st * STRIP
                n = len(grp)
                pool = expa if eng == "a" else expv
                eta = pool.tile([128, 3 * STRIP], BF16, tag="exp")
                if eng == "a":
                    nc.scalar.activation(
                        eta[:, : n * STRIP], sc[:, : n * STRIP], EXP, scale=ACT_SCALE
                    )
                else:
                    et1 = expi.tile([128, 3 * STRIP], BF16, tag="exp1")
                    et2 = expi.tile([128, 3 * STRIP], BF16, tag="exp2")
                    nc.vector.tensor_scalar(
                        et1.bitcast(I16)[:, : n * STRIP],
                        sc[:, : n * STRIP],
                        B2 - 128.0,
                        None,
                        op0=mybir.AluOpType.add,
                    )
                    nc.vector.tensor_scalar(
                        et2.bitcast(I16)[:, : n * STRIP],
                        sc[:, : n * STRIP],
                        B2 - 192.0,
                        None,
                        op0=mybir.AluOpType.add,
                    )
                    # eta = eta1 + sqrt(2)*eta2: averages the two sawtooths
                    et3 = expi.tile([128, 3 * STRIP], BF16, tag="exp3")
                    et4 = expi.tile([128, 3 * STRIP], BF16, tag="exp4")
                    nc.vector.tensor_scalar_mul(
                        et3[:, : n * STRIP], et2[:, : n * STRIP], SQRT2
                    )
                    nc.vector.tensor_add(
                        et4[:, : n * STRIP], et1[:, : n * STRIP], et3[:, : n * STRIP]
                    )
                    nc.vector.tensor_copy(eta[:, : n * STRIP], et4[:, : n * STRIP])
                return eta

            def issue_av(h, st, grp, eta, ot):
                _, _, vt = head_tiles[h]
                for i, kb in enumerate(grp):
                    nc.tensor.matmul(
                        ot[:],
                        vt[:, kb * (D + 1) : (kb + 1) * (D + 1)],
                        eta[:, i * STRIP : (i + 1) * STRIP],
                        start=(kb == AV_FIRST),
                        stop=(kb == AV_LAST),
                        skip_group_check=True,
                    )

            def issue_tail(h, st, ot):
                # ---- evacuate + transpose (PE-gates the ot bank reuse) ----
                ot_sb = otp.tile([D + 1, STRIP], FP32, tag="ot_sb")
                nc.scalar.copy(ot_sb[:], ot[:])
                ptt = ps_t.tile([128, NQT * (D + 1)], FP32, tag="tr")
                ptt3 = ptt.rearrange("p (n c) -> p n c", c=D + 1)
                for j in range(NQT):
                    nc.tensor.transpose(
                        ptt3[:, j],
                        ot_sb[:, j * 128 : (j + 1) * 128],
                        ident[: D + 1, : D + 1],
                    )
                return ptt3

            def issue_norm(h, st, ptt3):
                # ---- normalize + emit strip (deferred: keeps the DVE FIFO
                # clear of instructions gated on late PE pops) ----
                q0 = st * STRIP
                stg = stgp.tile([128, NQT * D], FP32, tag="stg")
                rec = smallp.tile([128, NQT], FP32, tag="rec")
                nc.vector.reciprocal(rec[:], ptt3[:, :, D])
                for j in range(NQT):
                    nc.vector.tensor_scalar_mul(
                        stg[:, j * D : (j + 1) * D],
                        ptt3[:, j, 0:D],
                        rec[:, j : j + 1],
                    )
                nc.sync.dma_start(
                    out_d[h, q0 : q0 + STRIP].rearrange("(n p) d -> p n d", p=128),
                    stg.rearrange("p (n d) -> p n d", d=D),
                )

            # software pipeline: scores(k+1) before AV(k-1); AV lags two steps
            # so strip tails (DVE ot copy) never stall the single-buffer ot
            pending = []  # [(h, st, grp, eta, ot, is_last, eng, age), ...]
            norms = []  # [(h, st, ptt3, age), ...]
            ot = None

            def pop_pending(flush=False):
                # 'a' etas (ACT-written) are safe for the PE at 2 steps of
                # lag; DVE-written 'v' etas need 4 steps of margin
                done = []
                i = 0
                while i < len(pending):
                    e = pending[i]
                    depth = 4 if e[6] == "v" else 2
                    if flush or e[7] >= depth:
                        pending.pop(i)
                        done.append(e)
                    else:
                        i += 1
                for i in range(len(pending)):
                    pending[i] = pending[i][:7] + (pending[i][7] + 1,)
                for ph, pst, pgrp, peta, pot, plast, _, _ in done:
                    issue_av(ph, pst, pgrp, peta, pot)
                    if plast:
                        norms.append((ph, pst, issue_tail(ph, pst, pot), 0))

            def pop_norms(flush=False):
                while norms and (flush or norms[0][3] >= 3):
                    nh, nst, nptt3, _ = norms.pop(0)
                    issue_norm(nh, nst, nptt3)
                for i in range(len(norms)):
                    norms[i] = (norms[i][0], norms[i][1], norms[i][2], norms[i][3] + 1)

            for h, st, gi, grp, eng in steps:
                if gi == 0 and st == 1 and h + 1 < HPC:
                    load_head(h + 1)
                if gi == 0:
                    ot = ps_o.tile([D + 1, STRIP], FP32, tag="ot")
                sc = issue_scores(h, st, grp)
                pop_pending()
                eta = issue_exp(h, st, grp, eng, sc)
                pending.append((h, st, grp, eta, ot, gi == len(GRPS) - 1, eng, 0))
                pop_norms()
            pop_pending(flush=True)
            pop_norms(flush=True)

    nc.compile()
    return nc


def prepare_in_maps(inputs):
    Q = np.ascontiguousarray(inputs["Q"], dtype=np.float32).reshape(B * H, S, D)
    K = np.ascontiguousarray(inputs["K"], dtype=np.float32).reshape(B * H, S, D)
    V = np.ascontiguousarray(inputs["V"], dtype=np.float32).reshape(B * H, S, D)
    inv_t = float(
        1.0 / np.asarray(inputs["temperature"], dtype=np.float32).reshape(-1)[0]
    )
    # Schraudolph pre-scale: scores come out as s * 2^23*log2(e)/(T*65536)
    c = (1 << 23) * math.log2(math.e) * inv_t / 65536.0
    QT = np.ascontiguousarray(Q.transpose(0, 2, 1)).astype(ml_dtypes.bfloat16)
    KT = (np.ascontiguousarray(K.transpose(0, 2, 1)) * c).astype(ml_dtypes.bfloat16)
    V16 = V.astype(ml_dtypes.bfloat16)
    in_maps = [
        {
            "QT": QT[i * HPC : (i + 1) * HPC],
            "KT": KT[i * HPC : (i + 1) * HPC],
            "V": V16[i * HPC : (i + 1) * HPC],
        }
        for i in range(N_CORES)
    ]
    return inv_t, in_maps


def kernel(**inputs: np.ndarray) -> np.ndarray:
    _, in_maps = prepare_in_maps(inputs)
    nc = build_nc()
    res = run_bass_kernel_spmd(nc, in_maps, core_ids=list(range(N_CORES)))
    outs = [res.results[i]["out"] for i in range(N_CORES)]
    return np.concatenate(outs, axis=0).reshape(B, H, S, D)


if __name__ == "__main__":
    rng = np.random.default_rng(0)
    ins = {
        "Q": rng.standard_normal((B, H, S, D), dtype=np.float32),
        "K": rng.standard_normal((B, H, S, D), dtype=np.float32),
        "V": rng.standard_normal((B, H, S, D), dtype=np.float32),
        "temperature": np.full((1,), 8.0, dtype=np.float32),
    }
    out = kernel(**ins)
    print("out", out.shape, out.dtype, float(np.abs(out).mean()))


# revision 25
# speedup vs baseline: 1.2978x; 1.2978x over previous
"""LSA attention (full S x S attention with diagonal self-exclusion) on 8 TRN2 cores.

Full inputs Q,K,V [4,12,2048,64] f32; heads flattened to 48 and split 6 per core
(no cross-core communication). Host-side prep: K,Q transposed to [h, 64, S] bf16
(KT/QT), V bf16.

Per head, per 512-wide q strip, the 16 k-blocks of 128 are processed in groups
(1,3,3,3,3,3) so the ACT engine (the bottleneck: exp at 1 elem/lane/cycle
@1.2GHz) runs one activation instruction per group, amortizing the ~350-cycle
ACT instruction overhead over N up to 1536. Score matmuls are row-packed: KT/QT
duplicated to partitions 64-127, adjacent k-blocks run on PE array row groups
0/64 concurrently (contract dim is 64). The diagonal is masked ON THE PE by
accumulating -6000*I onto each diagonal score block (one extra N=128 matmul;
exp then underflows to 0) - no cross-engine mask traffic. AV accumulates
out^T[65,q] in PSUM via V' tiles carrying a ones column (row 64 = softmax
denominators). Strip tail: DVE copy to SBUF, 4 PE transposes into one PSUM
bank, reciprocal + scale on DVE, DMA out.

Software pipeline: per step, scores(k) issue, then AV(k-1) (AV(k-2) for the
first group of each strip, giving the previous strip's ot-evacuation copy a
full extra step so the single-buffered ot bank never stalls the PE), then
exp(k). The in-order PE queue plus the AV(k)<-exp(k) RAW edge transitively
orders every PSUM-buffer reuse behind its readers.
"""

import sys

for _p in ("/opt/trn_rl_repo",):
    if _p not in sys.path:
        sys.path.insert(0, _p)

import ml_dtypes
import numpy as np

import concourse.bass as bass  # noqa: F401  (registers trn types)
import concourse.bacc as bacc
import concourse.mybir as mybir
import concourse.tile as tile
from concourse.bass_utils import run_bass_kernel_spmd
from concourse.masks import make_identity

N_CORES = 8
B, H, S, D = 4, 12, 2048, 64
HPC = (B * H) // N_CORES  # heads per core = 6
NKB = S // 128  # 16 k-blocks of 128
STRIP = 512
NSTRIP = S // STRIP  # 4 q strips per head
NQT = STRIP // 128  # 4 q-tiles per strip
# remainder group first: the strip's first exp needs only one score matmul,
# shortening the strip-boundary critical path
GRPS = [(15,), (0, 1, 2), (3, 4, 5), (6, 7, 8), (9, 10, 11), (12, 13, 14)]
AV_FIRST = 15
AV_LAST = 14
FP32 = mybir.dt.float32
BF16 = mybir.dt.bfloat16
EXP = mybir.ActivationFunctionType.Exp


def build_nc(inv_temp: float):
    nc = bacc.Bacc(None, target_bir_lowering=False)
    qt_d = nc.dram_tensor("QT", [HPC, D, S], BF16, kind="ExternalInput")
    kt_d = nc.dram_tensor("KT", [HPC, D, S], BF16, kind="ExternalInput")
    v_d = nc.dram_tensor("V", [HPC, S, D], BF16, kind="ExternalInput")
    out_d = nc.dram_tensor("out", [HPC, S, D], FP32, kind="ExternalOutput")

    with tile.TileContext(nc) as tc:
        with (
            tc.tile_pool(name="consts", bufs=1) as constp,
            tc.tile_pool(name="tr", bufs=2) as trp,
            tc.tile_pool(name="vpool", bufs=2) as vpool,
            tc.tile_pool(name="expp", bufs=4) as expp,
            tc.tile_pool(name="otsb", bufs=2) as otp,
            tc.tile_pool(name="stage", bufs=2) as stgp,
            tc.tile_pool(name="small", bufs=2) as smallp,
            tc.tile_pool(name="ps_s", bufs=2, space="PSUM") as ps_s,
            tc.tile_pool(name="ps_o", bufs=1, space="PSUM") as ps_o,
            tc.tile_pool(name="ps_t", bufs=1, space="PSUM") as ps_t,
        ):
            ident = constp.tile([128, 128], FP32)
            make_identity(nc, ident[:])
            idb = constp.tile([128, 128], BF16)
            nc.vector.tensor_copy(idb[:], ident[:])
            # -6000*I: accumulated onto diagonal score blocks by the PE itself;
            # exp((sc-6000)/T) underflows to 0
            negbig = constp.tile([128, 128], BF16)
            nc.vector.tensor_scalar_mul(negbig[:], idb[:], -6000.0)

            # preload the exp table set (one-time ~2.7us, overlaps warmup)
            tldin = constp.tile([128, 1], FP32)
            tldout = constp.tile([128, 1], FP32)
            nc.vector.memset(tldin[:], 0.0)
            nc.scalar.activation(tldout[:], tldin[:], EXP)

            # PE warmup: ~5us of dummy matmuls (>= one 3.4us HAM window) so the
            # clock gate opens to K=8/8 during the head-0 DMAs
            wsrc = constp.tile([128, 256], BF16, tag="wsrc")
            nc.vector.memset(wsrc[:], 0.5)
            for _w in range(24):
                wt = ps_t.tile([128, 256], FP32, tag="tr")
                nc.tensor.matmul(wt[:], idb[:], wsrc[:], start=True, stop=True)

            head_tiles = {}

            def load_head(h):
                # KT/QT [64, S] bf16, duplicated to partitions 64-127 so the
                # odd k-blocks' score matmuls run on PE array row group 64
                kt2 = trp.tile([128, S], BF16, tag="kt")
                nc.sync.dma_start(kt2[0:64, :], kt_d[h])
                nc.sync.dma_start(kt2[64:128, :], kt_d[h])
                qt2 = trp.tile([128, S], BF16, tag="qt")
                nc.sync.dma_start(qt2[0:64, :], qt_d[h])
                nc.sync.dma_start(qt2[64:128, :], qt_d[h])
                # V' tiles [128, 65] per k-block: V rows + ones column
                vt = vpool.tile([128, NKB * (D + 1)], BF16, tag="vt")
                vt3 = vt.rearrange("p (n c) -> p n c", c=D + 1)
                nc.sync.dma_start(
                    vt3[:, :, 0:D], v_d[h].rearrange("(n p) d -> p n d", p=128)
                )
                nc.vector.memset(vt3[:, :, D : D + 1], 1.0)
                head_tiles[h] = (kt2, qt2, vt)

            load_head(0)

            steps = []
            for h in range(HPC):
                for st in range(NSTRIP):
                    for gi, grp in enumerate(GRPS):
                        steps.append((h, st, gi, grp))

            def issue_scores(h, st, grp):
                kt2, qt2, _ = head_tiles[h]
                q0 = st * STRIP
                sc = ps_s.tile([128, 3 * STRIP], FP32, tag="sc")
                for i, kb in enumerate(grp):
                    rg = 64 * (i % 2)  # alternate row groups -> concurrent MMs
                    nc.tensor.matmul(
                        sc[:, i * STRIP : (i + 1) * STRIP],
                        kt2[rg : rg + 64, kb * 128 : (kb + 1) * 128],
                        qt2[rg : rg + 64, q0 : q0 + STRIP],
                        start=True,
                        stop=True,
                        skip_group_check=True,
                    )
                    if q0 <= kb * 128 < q0 + STRIP:
                        off = i * STRIP + kb * 128 - q0
                        nc.tensor.matmul(
                            sc[:, off : off + 128],
                            idb[:],
                            negbig[:],
                            start=False,
                            stop=True,
                            skip_group_check=True,
                        )
                return sc

            def issue_exp(h, st, grp, sc):
                n = len(grp)
                eta = expp.tile([128, 3 * STRIP], BF16, tag="exp")
                nc.scalar.activation(
                    eta[:, : n * STRIP], sc[:, : n * STRIP], EXP, scale=inv_temp
                )
                return eta

            def issue_av(h, st, grp, eta, ot):
                _, _, vt = head_tiles[h]
                for i, kb in enumerate(grp):
                    nc.tensor.matmul(
                        ot[:],
                        vt[:, kb * (D + 1) : (kb + 1) * (D + 1)],
                        eta[:, i * STRIP : (i + 1) * STRIP],
                        start=(kb == AV_FIRST),
                        stop=(kb == AV_LAST),
                        skip_group_check=True,
                    )

            def issue_tail(h, st, ot):
                # ---- normalize + emit strip ----
                q0 = st * STRIP
                ot_sb = otp.tile([D + 1, STRIP], FP32, tag="ot_sb")
                nc.vector.tensor_copy(ot_sb[:], ot[:])
                ptt = ps_t.tile([128, NQT * (D + 1)], FP32, tag="tr")
                ptt3 = ptt.rearrange("p (n c) -> p n c", c=D + 1)
                for j in range(NQT):
                    nc.tensor.transpose(
                        ptt3[:, j],
                        ot_sb[:, j * 128 : (j + 1) * 128],
                        ident[: D + 1, : D + 1],
                    )
                stg = stgp.tile([128, NQT * D], FP32, tag="stg")
                rec = smallp.tile([128, NQT], FP32, tag="rec")
                nc.vector.reciprocal(rec[:], ptt3[:, :, D])
                for j in range(NQT):
                    nc.vector.tensor_scalar_mul(
                        stg[:, j * D : (j + 1) * D],
                        ptt3[:, j, 0:D],
                        rec[:, j : j + 1],
                    )
                nc.sync.dma_start(
                    out_d[h, q0 : q0 + STRIP].rearrange("(n p) d -> p n d", p=128),
                    stg.rearrange("p (n d) -> p n d", d=D),
                )

            # pipeline: scores(k); AV(k-1) -- except each strip's FIRST group's
            # AV waits one extra step (depth 2) so the previous strip's
            # ot-evacuation never stalls the PE; then exp(k)
            pending = []  # [(h, st, grp, eta, ot, is_last, steps_left)]
            ot = None
            for h, st, gi, grp in steps:
                if gi == 0 and st == 1 and h + 1 < HPC:
                    load_head(h + 1)
                if gi == 0:
                    ot = ps_o.tile([D + 1, STRIP], FP32, tag="ot")
                sc = issue_scores(h, st, grp)
                for i in range(len(pending)):
                    pending[i] = pending[i][:6] + (pending[i][6] - 1,)
                while pending and pending[0][6] <= 0:
                    ph, pst, pgrp, peta, pot, plast, _ = pending.pop(0)
                    issue_av(ph, pst, pgrp, peta, pot)
                    if plast:
                        issue_tail(ph, pst, pot)
                eta = issue_exp(h, st, grp, sc)
                pending.append(
                    (h, st, grp, eta, ot, gi == len(GRPS) - 1, 2 if gi == 0 else 1)
                )
            for ph, pst, pgrp, peta, pot, plast, _ in pending:
                issue_av(ph, pst, pgrp, peta, pot)
                if plast:
                    issue_tail(ph, pst, pot)

    nc.compile()
    return nc


def prepare_in_maps(inputs):
    Q = np.ascontiguousarray(inputs["Q"], dtype=np.float32).reshape(B * H, S, D)
    K = np.ascontiguousarray(inputs["K"], dtype=np.float32).reshape(B * H, S, D)
    V = np.ascontiguousarray(inputs["V"], dtype=np.float32).reshape(B * H, S, D)
    inv_t = float(
        1.0 / np.asarray(inputs["temperature"], dtype=np.float32).reshape(-1)[0]
    )
    QT = np.ascontiguousarray(Q.transpose(0, 2, 1)).astype(ml_dtypes.bfloat16)
    KT = np.ascontiguousarray(K.transpose(0, 2, 1)).astype(ml_dtypes.bfloat16)
    V16 = V.astype(ml_dtypes.bfloat16)
    in_maps = [
        {
            "QT": QT[i * HPC : (i + 1) * HPC],
            "KT": KT[i * HPC : (i + 1) * HPC],
            "V": V16[i * HPC : (i + 1) * HPC],
        }
        for i in range(N_CORES)
    ]
    return inv_t, in_maps


def kernel(**inputs: np.ndarray) -> np.ndarray:
    inv_t, in_maps = prepare_in_maps(inputs)
    nc = build_nc(inv_t)
    res = run_bass_kernel_spmd(nc, in_maps, core_ids=list(range(N_CORES)))
    outs = [res.results[i]["out"] for i in range(N_CORES)]
    return np.concatenate(outs, axis=0).reshape(B, H, S, D)


if __name__ == "__main__":
    rng = np.random.default_rng(0)
    ins = {
        "Q": rng.standard_normal((B, H, S, D), dtype=np.float32),
        "K": rng.standard_normal((B, H, S, D), dtype=np.float32),
        "V": rng.standard_normal((B, H, S, D), dtype=np.float32),
        "temperature": np.full((1,), 8.0, dtype=np.float32),
    }
    out = kernel(**ins)
    print("out", out.shape, out.dtype, float(np.abs(out).mean()))


# revision 26
# speedup vs baseline: 1.3034x; 1.0043x over previous
"""LSA attention (full S x S attention with diagonal self-exclusion) on 8 TRN2 cores.

Full inputs Q,K,V [4,12,2048,64] f32; heads flattened to 48 and split 6 per core
(no cross-core communication). Host-side prep: K,Q transposed to [h, 64, S] bf16
(KT/QT), V bf16.

Per head, per 512-wide q strip, the 16 k-blocks of 128 are processed in groups
(1,3,3,3,3,3) so the ACT engine (the bottleneck: exp at 1 elem/lane/cycle
@1.2GHz) runs one activation instruction per group, amortizing the ~350-cycle
ACT instruction overhead over N up to 1536. Score matmuls are row-packed: KT/QT
duplicated to partitions 64-127, adjacent k-blocks run on PE array row groups
0/64 concurrently (contract dim is 64). The diagonal is masked ON THE PE by
accumulating -6000*I onto each diagonal score block (one extra N=128 matmul;
exp then underflows to 0) - no cross-engine mask traffic. AV accumulates
out^T[65,q] in PSUM via V' tiles carrying a ones column (row 64 = softmax
denominators). Strip tail: DVE copy to SBUF, 4 PE transposes into one PSUM
bank, reciprocal + scale on DVE, DMA out.

Software pipeline: per step, scores(k) issue, then AV(k-1) (AV(k-2) for the
first group of each strip, giving the previous strip's ot-evacuation copy a
full extra step so the single-buffered ot bank never stalls the PE), then
exp(k). The in-order PE queue plus the AV(k)<-exp(k) RAW edge transitively
orders every PSUM-buffer reuse behind its readers.
"""

import sys

for _p in ("/opt/trn_rl_repo",):
    if _p not in sys.path:
        sys.path.insert(0, _p)

import ml_dtypes
import numpy as np

import concourse.bass as bass  # noqa: F401  (registers trn types)
import concourse.bacc as bacc
import concourse.mybir as mybir
import concourse.tile as tile
from concourse.bass_utils import run_bass_kernel_spmd
from concourse.masks import make_identity

N_CORES = 8
B, H, S, D = 4, 12, 2048, 64
HPC = (B * H) // N_CORES  # heads per core = 6
NKB = S // 128  # 16 k-blocks of 128
STRIP = 512
NSTRIP = S // STRIP  # 4 q strips per head
NQT = STRIP // 128  # 4 q-tiles per strip
# remainder group first: the strip's first exp needs only one score matmul,
# shortening the strip-boundary critical path
GRPS = [(15,), (0, 1, 2), (3, 4, 5), (6, 7, 8), (9, 10, 11), (12, 13, 14)]
AV_FIRST = 15
AV_LAST = 14
FP32 = mybir.dt.float32
BF16 = mybir.dt.bfloat16
EXP = mybir.ActivationFunctionType.Exp


def build_nc(inv_temp: float):
    nc = bacc.Bacc(None, target_bir_lowering=False)
    qt_d = nc.dram_tensor("QT", [HPC, D, S], BF16, kind="ExternalInput")
    kt_d = nc.dram_tensor("KT", [HPC, D, S], BF16, kind="ExternalInput")
    v_d = nc.dram_tensor("V", [HPC, S, D], BF16, kind="ExternalInput")
    out_d = nc.dram_tensor("out", [HPC, S, D], FP32, kind="ExternalOutput")

    with tile.TileContext(nc) as tc:
        with (
            tc.tile_pool(name="consts", bufs=1) as constp,
            tc.tile_pool(name="tr", bufs=2) as trp,
            tc.tile_pool(name="vpool", bufs=2) as vpool,
            tc.tile_pool(name="expp", bufs=8) as expp,
            tc.tile_pool(name="otsb", bufs=2) as otp,
            tc.tile_pool(name="stage", bufs=2) as stgp,
            tc.tile_pool(name="small", bufs=2) as smallp,
            tc.tile_pool(name="ps_s", bufs=2, space="PSUM") as ps_s,
            tc.tile_pool(name="ps_o", bufs=1, space="PSUM") as ps_o,
            tc.tile_pool(name="ps_t", bufs=1, space="PSUM") as ps_t,
        ):
            ident = constp.tile([128, 128], FP32)
            make_identity(nc, ident[:])
            idb = constp.tile([128, 128], BF16)
            nc.vector.tensor_copy(idb[:], ident[:])
            # -6000*I: accumulated onto diagonal score blocks by the PE itself;
            # exp((sc-6000)/T) underflows to 0
            negbig = constp.tile([128, 128], BF16)
            nc.vector.tensor_scalar_mul(negbig[:], idb[:], -6000.0)

            # preload the exp table set (one-time ~2.7us, overlaps warmup)
            tldin = constp.tile([128, 1], FP32)
            tldout = constp.tile([128, 1], FP32)
            nc.vector.memset(tldin[:], 0.0)
            nc.scalar.activation(tldout[:], tldin[:], EXP)

            # PE warmup: ~5us of dummy matmuls (>= one 3.4us HAM window) so the
            # clock gate opens to K=8/8 during the head-0 DMAs
            wsrc = constp.tile([128, 256], BF16, tag="wsrc")
            nc.vector.memset(wsrc[:], 0.5)
            for _w in range(24):
                wt = ps_t.tile([128, 256], FP32, tag="tr")
                nc.tensor.matmul(wt[:], idb[:], wsrc[:], start=True, stop=True)

            head_tiles = {}

            def load_head(h):
                # KT/QT [64, S] bf16, duplicated to partitions 64-127 so the
                # odd k-blocks' score matmuls run on PE array row group 64
                kt2 = trp.tile([128, S], BF16, tag="kt")
                nc.sync.dma_start(kt2[0:64, :], kt_d[h])
                nc.sync.dma_start(kt2[64:128, :], kt_d[h])
                qt2 = trp.tile([128, S], BF16, tag="qt")
                nc.sync.dma_start(qt2[0:64, :], qt_d[h])
                nc.sync.dma_start(qt2[64:128, :], qt_d[h])
                # V' tiles [128, 65] per k-block: V rows + ones column
                vt = vpool.tile([128, NKB * (D + 1)], BF16, tag="vt")
                vt3 = vt.rearrange("p (n c) -> p n c", c=D + 1)
                nc.sync.dma_start(
                    vt3[:, :, 0:D], v_d[h].rearrange("(n p) d -> p n d", p=128)
                )
                nc.vector.memset(vt3[:, :, D : D + 1], 1.0)
                head_tiles[h] = (kt2, qt2, vt)

            load_head(0)

            steps = []
            for h in range(HPC):
                for st in range(NSTRIP):
                    for gi, grp in enumerate(GRPS):
                        steps.append((h, st, gi, grp))

            def issue_scores(h, st, grp):
                kt2, qt2, _ = head_tiles[h]
                q0 = st * STRIP
                sc = ps_s.tile([128, 3 * STRIP], FP32, tag="sc")
                for i, kb in enumerate(grp):
                    rg = 64 * (i % 2)  # alternate row groups -> concurrent MMs
                    nc.tensor.matmul(
                        sc[:, i * STRIP : (i + 1) * STRIP],
                        kt2[rg : rg + 64, kb * 128 : (kb + 1) * 128],
                        qt2[rg : rg + 64, q0 : q0 + STRIP],
                        start=True,
                        stop=True,
                        skip_group_check=True,
                    )
                    if q0 <= kb * 128 < q0 + STRIP:
                        off = i * STRIP + kb * 128 - q0
                        nc.tensor.matmul(
                            sc[:, off : off + 128],
                            idb[:],
                            negbig[:],
                            start=False,
                            stop=True,
                            skip_group_check=True,
                        )
                return sc

            def issue_exp(h, st, grp, sc):
                n = len(grp)
                eta = expp.tile([128, 3 * STRIP], BF16, tag="exp")
                nc.scalar.activation(
                    eta[:, : n * STRIP], sc[:, : n * STRIP], EXP, scale=inv_temp
                )
                return eta

            def issue_av(h, st, grp, eta, ot):
                _, _, vt = head_tiles[h]
                for i, kb in enumerate(grp):
                    nc.tensor.matmul(
                        ot[:],
                        vt[:, kb * (D + 1) : (kb + 1) * (D + 1)],
                        eta[:, i * STRIP : (i + 1) * STRIP],
                        start=(kb == AV_FIRST),
                        stop=(kb == AV_LAST),
                        skip_group_check=True,
                    )

            def issue_tail(h, st, ot):
                # ---- normalize + emit strip ----
                q0 = st * STRIP
                ot_sb = otp.tile([D + 1, STRIP], FP32, tag="ot_sb")
                nc.vector.tensor_copy(ot_sb[:], ot[:])
                ptt = ps_t.tile([128, NQT * (D + 1)], FP32, tag="tr")
                ptt3 = ptt.rearrange("p (n c) -> p n c", c=D + 1)
                for j in range(NQT):
                    nc.tensor.transpose(
                        ptt3[:, j],
                        ot_sb[:, j * 128 : (j + 1) * 128],
                        ident[: D + 1, : D + 1],
                    )
                stg = stgp.tile([128, NQT * D], FP32, tag="stg")
                rec = smallp.tile([128, NQT], FP32, tag="rec")
                nc.vector.reciprocal(rec[:], ptt3[:, :, D])
                for j in range(NQT):
                    nc.vector.tensor_scalar_mul(
                        stg[:, j * D : (j + 1) * D],
                        ptt3[:, j, 0:D],
                        rec[:, j : j + 1],
                    )
                nc.sync.dma_start(
                    out_d[h, q0 : q0 + STRIP].rearrange("(n p) d -> p n d", p=128),
                    stg.rearrange("p (n d) -> p n d", d=D),
                )

            # pipeline: scores(k); AV(k-1) -- except each strip's FIRST group's
            # AV waits one extra step (depth 2) so the previous strip's
            # ot-evacuation never stalls the PE; then exp(k)
            pending = []  # [(h, st, grp, eta, ot, is_last, steps_left)]
            ot = None
            for h, st, gi, grp in steps:
                if gi == 0 and st == 1 and h + 1 < HPC:
                    load_head(h + 1)
                if gi == 0:
                    ot = ps_o.tile([D + 1, STRIP], FP32, tag="ot")
                sc = issue_scores(h, st, grp)
                for i in range(len(pending)):
                    pending[i] = pending[i][:6] + (pending[i][6] - 1,)
                while pending and pending[0][6] <= 0:
                    ph, pst, pgrp, peta, pot, plast, _ = pending.pop(0)
                    issue_av(ph, pst, pgrp, peta, pot)
                    if plast:
                        issue_tail(ph, pst, pot)
                eta = issue_exp(h, st, grp, sc)
                pending.append(
                    (h, st, grp, eta, ot, gi == len(GRPS) - 1, 2 if gi == 0 else 1)
                )
            for ph, pst, pgrp, peta, pot, plast, _ in pending:
                issue_av(ph, pst, pgrp, peta, pot)
                if plast:
                    issue_tail(ph, pst, pot)

    nc.compile()
    return nc


def prepare_in_maps(inputs):
    Q = np.ascontiguousarray(inputs["Q"], dtype=np.float32).reshape(B * H, S, D)
    K = np.ascontiguousarray(inputs["K"], dtype=np.float32).reshape(B * H, S, D)
    V = np.ascontiguousarray(inputs["V"], dtype=np.float32).reshape(B * H, S, D)
    inv_t = float(
        1.0 / np.asarray(inputs["temperature"], dtype=np.float32).reshape(-1)[0]
    )
    QT = np.ascontiguousarray(Q.transpose(0, 2, 1)).astype(ml_dtypes.bfloat16)
    KT = np.ascontiguousarray(K.transpose(0, 2, 1)).astype(ml_dtypes.bfloat16)
    V16 = V.astype(ml_dtypes.bfloat16)
    in_maps = [
        {
            "QT": QT[i * HPC : (i + 1) * HPC],
            "KT": KT[i * HPC : (i + 1) * HPC],
            "V": V16[i * HPC : (i + 1) * HPC],
        }
        for i in range(N_CORES)
    ]
    return inv_t, in_maps


def kernel(**inputs: np.ndarray) -> np.ndarray:
    inv_t, in_maps = prepare_in_maps(inputs)
    nc = build_nc(inv_t)
    res = run_bass_kernel_spmd(nc, in_maps, core_ids=list(range(N_CORES)))
    outs = [res.results[i]["out"] for i in range(N_CORES)]
    return np.concatenate(outs, axis=0).reshape(B, H, S, D)


if __name__ == "__main__":
    rng = np.random.default_rng(0)
    ins = {
        "Q": rng.standard_normal((B, H, S, D), dtype=np.float32),
        "K": rng.standard_normal((B, H, S, D), dtype=np.float32),
        "V": rng.standard_normal((B, H, S, D), dtype=np.float32),
        "temperature": np.full((1,), 8.0, dtype=np.float32),
    }
    out = kernel(**ins)
    print("out", out.shape, out.dtype, float(np.abs(out).mean()))
